# revision 24
# baseline (speedup 1.0000x reference)
"""Trainium2 kernel for nn_EuclideanEmbedding (edge-scale + segment_sum).

Computes: out[n, :] = inv * sum_{e: receivers[e]==n} sh_vectors[e, :] * cutoffs[e]

Distribution strategy (host side, inside kernel()):
  - Edges are sharded across the 8 NeuronCores BY RECEIVER NODE RANGE:
    core c owns nodes [c*6250, (c+1)*6250) and receives exactly the edges
    targeting those nodes.  Each core produces its own disjoint slice of the
    output, so the final "all-reduce" degenerates to a concatenation.
  - Within a core, edges are laid out DENSELY PER NODE: nodes are ordered by
    degree (descending) and split into 10 groups of 640; every node in group
    g gets a fixed budget of cap_g slots (max degree in the group, rounded
    up, SPMD-uniform across cores).  Node q's edges occupy slots [0, deg),
    the rest are zero-filled.  This turns the segment_sum scatter into a
    purely dense, race-free segmented reduction — no scatter-add DMA at all
    — while the degree sort keeps zero-padding small.
    (A dma_scatter_add variant was measured at 3.85 ms: SWDGE descriptor
    generation is ~7.5 ns/element on the Pool engine and the per-element CCE
    read-add-write costs ~150 ns of DMA-ring time.  The dense reduction with
    flat capacity measured 135 us; degree-sorted capacity cuts the padded
    volume further.)

Device program per core (identical SPMD program, different data):
  preload cutoffs (node-slot layout) and inv, fold inv into the cutoffs once;
  then per group: DMA the [128, 5*16*cap_g] sh slots (d-major per node so the
  slot reduction is contiguous), multiply by the broadcast cutoff (split
  between the Vector and GpSimd engines to balance load), tensor_reduce over
  the slot axis straight into the output tile; one DMA writes [6400, 16] out.
Sorted node position q maps to tile j = q // 128 (group g = j // 5),
partition p = q % 128; the host inverts the degree-sort permutation while
assembling the full output.
"""

import os

import numpy as np

# ---------------------------------------------------------------- constants
N_NODES = 50_000
D_SH = 16
N_CORES = 8
NPC = N_NODES // N_CORES          # 6250 nodes per core
NPAD = 6400                       # padded nodes per core
JTILES = NPAD // 128              # 50 node-tiles per core
JGRP = 5                          # node-tiles per group (per DVE op / DMA)
NGRP = JTILES // JGRP             # 10 groups of 640 nodes
CAP_Q = 4                         # capacity quantum
# every group's multiply is split along jj: Vector takes tiles [0, JV),
# GpSimd takes [JV, JGRP) — Vector also does all the reductions, GpSimd is
# ~2x slower per element, so 2/5 vs 3/5 roughly balances the engines.
JV = 2
GROUP_ORDER = tuple(reversed(range(NGRP)))

_NC_CACHE: dict = {}
LAST_RESULTS = None  # BassKernelResults of the most recent run (for test.py)


# ---------------------------------------------------------------- device IR
def build_nc(caps: tuple):
    """Build + compile the per-core Bass program for per-group slot
    capacities `caps` (len NGRP)."""
    key = tuple(caps)
    if key in _NC_CACHE:
        return _NC_CACHE[key]

    import concourse.bacc as bacc
    import concourse.bass as bass
    import concourse.mybir as mybir
    from concourse import tile

    nc = bacc.Bacc("TRN2", target_bir_lowering=False, debug=False)

    gcols = [JGRP * D_SH * c for c in caps]      # f32 per partition per group
    goffs = np.concatenate([[0], np.cumsum([128 * gc for gc in gcols])])
    cutcols = [JGRP * c for c in caps]
    cutoffs_off = np.concatenate([[0], np.cumsum(cutcols)])
    tot_cut = int(cutoffs_off[-1])

    sh = nc.dram_tensor("sh", [int(goffs[-1])], mybir.dt.float32,
                        kind="ExternalInput")
    cut = nc.dram_tensor("cut", [128, tot_cut], mybir.dt.float32,
                         kind="ExternalInput")
    inv = nc.dram_tensor("inv", [128, 1], mybir.dt.float32,
                         kind="ExternalInput")
    out = nc.dram_tensor("out", [NPAD, D_SH], mybir.dt.float32,
                         kind="ExternalOutput")

    with tile.TileContext(nc) as tc:
        with (
            tc.tile_pool(name="const", bufs=1) as cpool,
            tc.tile_pool(name="io", bufs=4) as pool,
            tc.tile_pool(name="redp", bufs=8) as rpool,
        ):
            inv_t = cpool.tile([128, 1], mybir.dt.float32)
            nc.sync.dma_start(inv_t[:], inv[:])
            cut_t = cpool.tile([128, tot_cut], mybir.dt.float32)
            nc.sync.dma_start(cut_t[:], cut[:])
            out_sb = cpool.tile([128, JTILES * D_SH], mybir.dt.float32)
            cut2 = cpool.tile([128, tot_cut], mybir.dt.float32)

            for g in GROUP_ORDER:
                cap = caps[g]
                gc = gcols[g]
                sh_t = pool.tile([128, gc], mybir.dt.float32, tag="sh")
                src = bass.AP(sh.ap().tensor, int(goffs[g]),
                              [[gc, 128], [1, gc]])
                nc.sync.dma_start(sh_t[:], src)

                # cut2_g = cut_g * inv
                lo, hi = int(cutoffs_off[g]), int(cutoffs_off[g + 1])
                c2 = cut2[:, lo:hi]
                ct = cut_t[:, lo:hi]
                inv_b = bass.AP(inv_t[:].tensor, inv_t[:].offset,
                                [list(inv_t[:].ap[0]), [0, hi - lo]])
                nc.vector.tensor_mul(c2, ct, inv_b)

                # scl[p, jj, d, s] = sh[p, jj, d, s] * cut2[p, jj*cap + s]
                # split along jj: Vector takes JV tiles, GpSimd the rest
                # (separate scl tiles so the two writers don't serialize)
                red = rpool.tile([128, JGRP * D_SH], mybir.dt.float32,
                                 tag="red")
                for eng, j0, j1, tg in ((nc.vector, 0, JV, "scla"),
                                        (nc.gpsimd, JV, JGRP, "sclb")):
                    nj = j1 - j0
                    scl = pool.tile([128, nj * D_SH * cap], mybir.dt.float32,
                                    tag=tg)
                    sh4 = bass.AP(sh_t[:].tensor,
                                  sh_t[:].offset + j0 * D_SH * cap,
                                  [list(sh_t[:].ap[0]), [D_SH * cap, nj],
                                   [cap, D_SH], [1, cap]])
                    scl4 = bass.AP(scl[:].tensor, scl[:].offset,
                                   [list(scl[:].ap[0]), [D_SH * cap, nj],
                                    [cap, D_SH], [1, cap]])
                    cut_b = bass.AP(c2.tensor, c2.offset + j0 * cap,
                                    [list(c2.ap[0]), [cap, nj], [0, D_SH],
                                     [1, cap]])
                    eng.tensor_mul(scl4, sh4, cut_b)
                    # red[p, (jj, d)] = sum_s scl[p, jj, d, s]
                    nc.vector.tensor_reduce(
                        red[:, j0 * D_SH:j1 * D_SH], scl4,
                        mybir.AxisListType.X, mybir.AluOpType.add)
                # collect on the (idle) scalar engine to avoid cross-group
                # serialization on a shared reduce target
                nc.scalar.mul(out_sb[:, g * JGRP * D_SH:(g + 1) * JGRP * D_SH],
                              red[:], 1.0)

            out3 = out_sb[:].rearrange("p (j d) -> p j d", d=D_SH)
            nc.sync.dma_start(out.ap().rearrange("(j p) d -> p j d", p=128),
                              out3)

    nc.compile()
    _NC_CACHE[key] = nc
    return nc


# ---------------------------------------------------------------- host shard
def shard_inputs(sh_vectors, cutoffs, receivers, inv_avg_num_neighbors):
    """Partition edges by receiver range, degree-sort nodes, build dense
    per-node slot layouts.  Returns (in_maps, caps, node_orders)."""
    sh_np = np.ascontiguousarray(np.asarray(sh_vectors, dtype=np.float32))
    cut_np = np.asarray(cutoffs, dtype=np.float32).ravel()
    rec = np.asarray(receivers).astype(np.int64).ravel()
    inv_val = np.float32(np.asarray(inv_avg_num_neighbors).ravel()[0])

    order = np.argsort(rec, kind="stable")       # sorts by (core, local)
    rec_sorted = rec[order]
    first = np.searchsorted(rec_sorted, rec_sorted, side="left")
    occ = np.arange(rec.size) - first            # occurrence within node
    bounds = np.searchsorted(rec_sorted, np.arange(0, N_NODES + 1, NPC))

    # per-core degree tables and degree-sorted node orders
    degs = np.zeros((N_CORES, NPAD), dtype=np.int64)
    node_orders = []
    pos_of_node = []
    for c in range(N_CORES):
        lseg = rec_sorted[bounds[c]:bounds[c + 1]] - c * NPC
        d = np.bincount(lseg, minlength=NPAD)
        degs[c] = d
        no = np.argsort(-d, kind="stable")       # position q -> node id
        node_orders.append(no)
        pon = np.empty(NPAD, dtype=np.int64)
        pon[no] = np.arange(NPAD)
        pos_of_node.append(pon)

    # per-group capacities: max degree among positions in the group,
    # maximized across cores, rounded up to CAP_Q
    gsz = JGRP * 128
    caps = []
    for g in range(NGRP):
        mx = 1
        for c in range(N_CORES):
            seg = degs[c][node_orders[c][g * gsz:(g + 1) * gsz]]
            if seg.size:
                mx = max(mx, int(seg.max()))
        caps.append(int(-(-mx // CAP_Q) * CAP_Q))
    caps = tuple(caps)

    gcols = [JGRP * D_SH * cp for cp in caps]
    goffs = np.concatenate([[0], np.cumsum([128 * gc for gc in gcols])])
    cutcols = [JGRP * cp for cp in caps]
    cutoffs_off = np.concatenate([[0], np.cumsum(cutcols)])
    tot_cut = int(cutoffs_off[-1])

    # per-(group) base offset helpers for a node position q:
    #   g = q // 640, j = q // 128, p = q % 128, jj = j - g*JGRP
    #   sh flat elem = goffs[g] + p*gcols[g] + jj*(16*cap) + d*cap + s
    #   cut col      = cutoffs_off[g] + jj*cap + s   (row p)
    in_maps = []
    inv_dev = np.full((128, 1), inv_val, dtype=np.float32)
    cap_arr = np.asarray(caps, dtype=np.int64)
    goffs_a = goffs.astype(np.int64)
    cutoffs_a = cutoffs_off.astype(np.int64)
    for c in range(N_CORES):
        lo, hi = bounds[c], bounds[c + 1]
        edges = order[lo:hi]
        l = rec_sorted[lo:hi] - c * NPC          # local node id, sorted
        o = occ[lo:hi]
        q = pos_of_node[c][l]                    # degree-sorted position
        g = q // (JGRP * 128)
        j = q // 128
        p = q - j * 128
        jj = j - g * JGRP
        cap_e = cap_arr[g]
        flat = (goffs_a[g] + p * (JGRP * D_SH * cap_e)
                + jj * (D_SH * cap_e) + o)       # d=0 element; d stride = cap
        sh_dev = np.zeros(int(goffs_a[-1]), dtype=np.float32)
        # write all 16 d-components with stride cap_e
        base = flat
        shv = sh_np[edges]
        for d in range(D_SH):
            sh_dev[base + d * cap_e] = shv[:, d]
        cut_dev = np.zeros((128, tot_cut), dtype=np.float32)
        cut_dev[p, cutoffs_a[g] + jj * cap_e + o] = cut_np[edges]
        in_maps.append({"sh": sh_dev, "cut": cut_dev, "inv": inv_dev})
    return in_maps, caps, node_orders


# ---------------------------------------------------------------- profiling
def _install_ntff_shim() -> bool:
    """This image's antenv lacks the axon_hooks shim that bass_utils imports
    for trace=True under axon.  Recreate it from trn_agent_boot's ctypes hook
    so NTFF profiling works.  Returns True on success."""
    try:
        import sys
        import types

        import antenv

        if getattr(antenv, "axon_hooks", None) is not None:
            return True
        import trn_agent_boot.trn_boot as tb

        hook = tb._ntff_profile_via_ctypes("/opt/axon/libaxon_pjrt.so")
        mod = types.ModuleType("antenv.axon_hooks")
        mod._hook = hook
        mod.get_axon_ntff_profile_hook = lambda: mod._hook
        mod.set_axon_ntff_profile_hook = lambda h: setattr(mod, "_hook", h)
        sys.modules["antenv.axon_hooks"] = mod
        antenv.axon_hooks = mod
        return hook is not None
    except Exception as e:  # profiling is best-effort; the run must not break
        print(f"ntff shim unavailable: {e!r}")
        return False


# ---------------------------------------------------------------- entrypoint
def kernel(sh_vectors, cutoffs, receivers, inv_avg_num_neighbors) -> np.ndarray:
    global LAST_RESULTS
    from concourse.bass_utils import run_bass_kernel_spmd

    in_maps, caps, node_orders = shard_inputs(sh_vectors, cutoffs, receivers,
                                              inv_avg_num_neighbors)
    nc = build_nc(caps)

    trace = os.environ.get("KERNEL_TRACE", "0") == "1"
    if trace:
        trace = _install_ntff_shim()
    res = run_bass_kernel_spmd(nc, in_maps, core_ids=list(range(N_CORES)),
                               trace=trace)
    LAST_RESULTS = res

    full = np.empty((N_NODES, D_SH), dtype=np.float32)
    for c in range(N_CORES):
        o = res.results[c]["out"]                # row q -> node node_orders[q]
        blk = np.empty((NPAD, D_SH), dtype=np.float32)
        blk[node_orders[c]] = o
        full[c * NPC:(c + 1) * NPC] = blk[:NPC]
    return full
